# revision 25
# baseline (speedup 1.0000x reference)
"""CrossAttention kernel for 8 TRN2 NeuronCores (Bass/Tile).

Reference computation (per batch b):
    q = x @ Wq ; k = ctx @ Wk ; v = ctx @ Wv        (heads H=8, dh=64)
    attn = softmax(q k^T / sqrt(dh)) ; o = attn @ v
    out = o @ Wo + bo

Sharding (8 cores): core c -> (batch b = c//2, head-group hg = c%2).
Each core handles 4 heads of one batch over the full sequence; the two
head-group partial outputs per batch are summed on the host (Wo is
sliced by rows, so partials add exactly).

Layout strategy: everything on-chip is kept "feature-major" (transposed)
so no on-chip transpose is ever needed:
  - host passes xT=[512,2048], cT=[512,2048] (bf16)
  - QT = Wq^T x^T, KT = Wk^T c^T  (d on partitions, seq on free)
  - V  = c @ Wv natural            (seq on partitions, d on free)
  - S^T = K_h Q_h^T                (keys m on partitions, queries n free)
  - E = exp(S^T/8)  (ScalarE, read PSUM directly; logits are tiny so no
    max-subtraction is needed -- softmax is shift-invariant)
  - O' = [V_h | ones]^T-style: V-matmul (M=64) plus a concurrent ones
    matmul (M=1, col group 2) accumulating the softmax denominators
  - O^T normalized with 1/sums, concatenated; Y^T = Wo_s^T O^T
"""

import os

import ml_dtypes
import numpy as np

import concourse.bass as bass
import concourse.mybir as mybir
import concourse.tile as tile
from concourse import bacc
from concourse.bass_utils import run_bass_kernel_spmd

BF16 = mybir.dt.bfloat16
F32 = mybir.dt.float32

D = 512          # model dim
N = 2048         # query seq len
M = 2048         # key seq len
HPC = 4          # heads per core
DH = 64          # head dim
DS = HPC * DH    # per-core inner dim = 256
SCALE = 1.0 / 8.0  # 1/sqrt(64)

_NBF = ml_dtypes.bfloat16


def _build_nc():
    nc = bacc.Bacc(None, target_bir_lowering=False)

    xT = nc.declare_dram_parameter("xT", [D, N], BF16, isOutput=False)
    cT = nc.declare_dram_parameter("cT", [D, M], BF16, isOutput=False)
    wq = nc.declare_dram_parameter("wq", [D, DS], BF16, isOutput=False)
    wk = nc.declare_dram_parameter("wk", [D, DS], BF16, isOutput=False)
    wv = nc.declare_dram_parameter("wv", [D, DS], BF16, isOutput=False)
    wo = nc.declare_dram_parameter("wo", [DS, D], BF16, isOutput=False)
    yT = nc.declare_dram_parameter("yT", [D, N], F32, isOutput=True)

    with tile.TileContext(nc) as tc:
        _emit(tc, xT, cT, wq, wk, wv, wo, yT)
    nc.finalize()
    return nc


def _emit(tc, xT, cT, wq, wk, wv, wo, yT):
    nc = tc.nc
    P = 128
    KT_D = D // P        # 4 k-tiles over model dim
    MT = M // P          # 16 m-tiles over keys
    NCH = 1024           # n-chunk for the attention inner loop
    EXP = mybir.ActivationFunctionType.Exp

    from contextlib import ExitStack

    with ExitStack() as ctx:
        const = ctx.enter_context(tc.tile_pool(name="const", bufs=1))
        work = ctx.enter_context(tc.tile_pool(name="work", bufs=4))
        yout = ctx.enter_context(tc.tile_pool(name="yout", bufs=2))
        ps_s = ctx.enter_context(tc.tile_pool(name="ps_s", bufs=2, space="PSUM"))
        ps_o = ctx.enter_context(tc.tile_pool(name="ps_o", bufs=3, space="PSUM"))
        ps_p = ctx.enter_context(tc.tile_pool(name="ps_p", bufs=1, space="PSUM"))

        # ---- resident SBUF tensors ----
        xT_sb = const.tile([P, KT_D, N], BF16)
        cT_sb = const.tile([P, KT_D, M], BF16)
        wq_sb = const.tile([P, KT_D, DS], BF16)
        wk_sb = const.tile([P, KT_D, DS], BF16)
        wv_sb = const.tile([P, KT_D, DS], BF16)
        wo_sb = const.tile([P, DS // P, D], BF16)
        QT_sb = const.tile([P, DS // P, N], BF16)
        KT_sb = const.tile([P, DS // P, M], BF16)
        # per (m-tile, head): 128 stationary columns = [V_h (64) | ones (64)]
        # so one matmul yields O'^T rows 0..63 AND the softmax sums
        # replicated on rows 64..127 (pre-broadcast for the normalize).
        Vp_sb = const.tile([P, MT, HPC, P], BF16)
        Ocat = const.tile([P, DS // P, N], BF16)

        # two DMA queues in parallel: sync carries the context path (gates
        # V/K projections), gpsimd carries the query path + output weights
        nc.sync.dma_start(cT_sb[:], cT.rearrange("(ko p) n -> p ko n", p=P))
        nc.sync.dma_start(wv_sb[:], wv.rearrange("(ko p) d -> p ko d", p=P))
        nc.sync.dma_start(wk_sb[:], wk.rearrange("(ko p) d -> p ko d", p=P))
        nc.gpsimd.dma_start(xT_sb[:], xT.rearrange("(ko p) n -> p ko n", p=P))
        nc.gpsimd.dma_start(wq_sb[:], wq.rearrange("(ko p) d -> p ko d", p=P))
        nc.gpsimd.dma_start(wo_sb[:], wo.rearrange("(ko p) d -> p ko d", p=P))
        nc.vector.memset(Vp_sb[:, :, :, DH:P], 1.0)

        # ---- just-in-time projections (emitted inside the attention loop
        # so PE reaches the first softmax chunk within a few microseconds
        # instead of draining all 160 projection matmuls first) ----
        proj_done = set()

        def emit_qk(w_sb, src_sb, dst_sb, dt, ch, key):
            if (key, dt, ch) in proj_done:
                return
            proj_done.add((key, dt, ch))
            ps = ps_p.tile([P, 512], F32, tag="psp", name=f"{key}{dt}{ch}")
            for kt in range(KT_D):
                nc.tensor.matmul(
                    ps[:, :512],
                    lhsT=w_sb[:, kt, dt * P:(dt + 1) * P],
                    rhs=src_sb[:, kt, ch * 512:(ch + 1) * 512],
                    start=(kt == 0),
                    stop=(kt == KT_D - 1),
                )
            nc.vector.tensor_copy(
                dst_sb[:, dt, ch * 512:(ch + 1) * 512], ps[:, :512]
            )

        def emit_v(mt):
            if ("v", mt) in proj_done:
                return
            proj_done.add(("v", mt))
            ps = ps_p.tile([P, 512], F32, tag="psp", name=f"v{mt}")
            for kt in range(KT_D):
                nc.tensor.matmul(
                    ps[:, :DS],
                    lhsT=cT_sb[:, kt, mt * P:(mt + 1) * P],
                    rhs=wv_sb[:, kt, :],
                    start=(kt == 0),
                    stop=(kt == KT_D - 1),
                )
            nc.vector.tensor_copy(
                Vp_sb[:, mt, :, 0:DH],
                ps[:, 0:DS].rearrange("p (h d) -> p h d", h=HPC),
            )

        # ---- attention + output projection, chunked over queries ----
        # V projection upfront (N=256 matmuls, feeds every block; keeping it
        # out of the attention loop avoids PE contention during block 0)
        for mt in range(MT):
            emit_v(mt)

        # head pairs (2p, 2p+1) live at partition offsets 0/64 of d-tile p:
        # their K=64 S-matmuls use disjoint PE row groups (concurrent), and
        # share one [128, 1024] PSUM tile -> a single 1024-wide exp.
        NS = 512  # n sub-chunk
        for nch in range(N // NS):
            n0 = nch * NS
            for pr in range(HPC // 2):
                h0, h1 = 2 * pr, 2 * pr + 1
                po = [
                    ps_o.tile([P, NS], F32, tag="po", name=f"po{i}")
                    for i in range(2)
                ]
                for mt in range(MT):
                    emit_qk(wk_sb, cT_sb, KT_sb, pr, mt // 4, "k")
                    emit_qk(wq_sb, xT_sb, QT_sb, pr, n0 // 512, "q")
                    # prefetch upcoming chunks into mid-block PE idle slots so
                    # they never gate the first S-matmul of a later block
                    if pr == 0 and mt % 4 == 2:
                        emit_qk(wk_sb, cT_sb, KT_sb, 1, mt // 4, "k")
                    if pr == 0 and mt == 6:
                        emit_qk(wq_sb, xT_sb, QT_sb, 1, n0 // 512, "q")
                    if pr == 1 and nch + 1 < N // NS:
                        if mt == 8:
                            emit_qk(wq_sb, xT_sb, QT_sb, 0, nch + 1, "q")
                        if mt == 10:
                            emit_qk(wq_sb, xT_sb, QT_sb, 1, nch + 1, "q")
                    st = ps_s.tile([P, NCH], F32, tag="ps")
                    for i, h in enumerate((h0, h1)):
                        dp = (h % 2) * DH
                        nc.tensor.matmul(
                            st[:, i * NS:(i + 1) * NS],
                            lhsT=KT_sb[dp:dp + DH, pr, mt * P:(mt + 1) * P],
                            rhs=QT_sb[dp:dp + DH, pr, n0:n0 + NS],
                            start=True,
                            stop=True,
                        )
                    e = work.tile([P, NCH], BF16, tag="e")
                    nc.scalar.activation(e[:], st[:], EXP, scale=SCALE)
                    for i, h in enumerate((h0, h1)):
                        nc.tensor.matmul(
                            po[i][:],
                            lhsT=Vp_sb[:, mt, h, :],
                            rhs=e[:, i * NS:(i + 1) * NS],
                            start=(mt == 0),
                            stop=(mt == MT - 1),
                        )
                # normalize: O^T = O'^T * (1/sums); sums already on rows 64..127
                for i, h in enumerate((h0, h1)):
                    dp = (h % 2) * DH
                    sc = work.tile([DH, NS], F32, tag="sc")
                    nc.vector.tensor_copy(sc[:], po[i][DH:P, :])
                    rc = work.tile([DH, NS], F32, tag="rc")
                    nc.vector.reciprocal_approx_fast(rc[:], sc[:])
                    nc.vector.tensor_tensor(
                        Ocat[dp:dp + DH, pr, n0:n0 + NS],
                        po[i][0:DH, :],
                        rc[:],
                        mybir.AluOpType.mult,
                    )

        # ---- Y^T = Wo_s^T O^T, emitted last: lowest PE priority, so these
        # matmuls fill TensorE idle slots during the ACT-bound attention ----
        for nch in range(N // NS):
            n0 = nch * NS
            yt = yout.tile([P, D // P, NS], F32, tag="y")
            for dt4 in range(D // P):
                ps = ps_p.tile([P, NS], F32, tag="psp")
                for kt in range(DS // P):
                    nc.tensor.matmul(
                        ps[:, :NS],
                        lhsT=wo_sb[:, kt, dt4 * P:(dt4 + 1) * P],
                        rhs=Ocat[:, kt, n0:n0 + NS],
                        start=(kt == 0),
                        stop=(kt == DS // P - 1),
                    )
                nc.vector.tensor_copy(yt[:, dt4, :], ps[:, :NS])
            nc.sync.dma_start(
                yT.rearrange("(dt p) n -> p dt n", p=P)[:, :, n0:n0 + NS],
                yt[:],
            )


def _install_ntff_hook():
    """Best-effort NTFF profiling under axon: provide the antenv.axon_hooks
    shim the boot code looks for, and avoid the artifact upload."""
    try:
        import sys
        import types

        import concourse.bass_utils as bu

        bu.upload_artifacts = lambda d: d  # no S3 in this sandbox
        try:
            from antenv.axon_hooks import get_axon_ntff_profile_hook  # noqa: F401
            return  # already present
        except ImportError:
            pass
        import antenv
        from trn_agent_boot.trn_boot import _ntff_profile_via_ctypes

        mod = types.ModuleType("antenv.axon_hooks")
        _state = {"hook": _ntff_profile_via_ctypes("/opt/axon/libaxon_pjrt.so")}
        mod.set_axon_ntff_profile_hook = lambda h: _state.__setitem__("hook", h)
        mod.get_axon_ntff_profile_hook = lambda: _state["hook"]
        sys.modules["antenv.axon_hooks"] = mod
        antenv.axon_hooks = mod
    except Exception as e:  # pragma: no cover
        print(f"ntff hook install failed ({e}); running without trace")


def kernel(x, context, Wq, Wk, Wv, Wo, bo):
    x = np.asarray(x, dtype=np.float32)
    context = np.asarray(context, dtype=np.float32)
    Wq = np.asarray(Wq, dtype=np.float32)
    Wk = np.asarray(Wk, dtype=np.float32)
    Wv = np.asarray(Wv, dtype=np.float32)
    Wo = np.asarray(Wo, dtype=np.float32)
    bo = np.asarray(bo, dtype=np.float32)
    B = x.shape[0]

    in_maps = []
    for c in range(8):
        b, hg = c // 2, c % 2
        sl = slice(hg * DS, (hg + 1) * DS)
        in_maps.append({
            "xT": np.ascontiguousarray(x[b].T).astype(_NBF),
            "cT": np.ascontiguousarray(context[b].T).astype(_NBF),
            "wq": np.ascontiguousarray(Wq[:, sl]).astype(_NBF),
            "wk": np.ascontiguousarray(Wk[:, sl]).astype(_NBF),
            "wv": np.ascontiguousarray(Wv[:, sl]).astype(_NBF),
            "wo": np.ascontiguousarray(Wo[sl, :]).astype(_NBF),
        })

    nc = _build_nc()
    trace = bool(int(os.environ.get("BASS_KERNEL_TRACE", "0")))
    if trace:
        _install_ntff_hook()
    res = run_bass_kernel_spmd(nc, in_maps, list(range(8)), trace=trace)
    if trace and res.exec_time_ns is not None:
        print(f"HW exec time: {res.exec_time_ns} ns")

    out = np.empty((B, N, D), dtype=np.float32)
    for b in range(B):
        yt = res.results[2 * b]["yT"] + res.results[2 * b + 1]["yT"]
        out[b] = yt.T + bo[None, :]
    return out
